# revision 1
# baseline (speedup 1.0000x reference)
"""CrossModalAttention kernel for 8 Trainium2 NeuronCores.

Data-parallel over batch: B=16 -> 2 batches per core.

Math (per batch, with A=audio [N,d], B=video [M,d]):
  scores*sqrt(d) = (A Wa^T + 1 b_a^T)(B Wv^T + 1 b_v^T)^T
                 = A M B^T + (row-constant terms) + 1_n w^T
  with M = Wa^T Wv, w = B (Wv^T b_a).  Row-constant terms drop inside
  softmax, and max-subtraction is skipped: scores are ~N(0,1), far from
  fp32 exp overflow.
  attn     = exp_s / rowsum, with exp_s kept transposed [m, n]
  att_T[d,n] = sum_m video[m,d] exp_s[m,n]
  out[n,f] = (att_T^T @ Wo^T + rowsum[n]*b_o) * (1/rowsum[n])

All matmuls run as fp32r (TF32-like, 1 cyc/row) with fp32 PSUM
accumulation; softmax internals stay fp32.  K is always on partitions:

  aT[d1,n]  <- PE-transpose of audio slice
  gT[d2,n]  =  M[d1,d2](st)       x aT(mv)
  sT[m,n]   =  videoT[d2,m](st)   x gT(mv);   exp on ACT
  rs[1,n]   =  ones[m,1](st)      x exp(mv)
  attT[d,n] =  video_r[m,d](st)   x exp(mv)
  out[n,f]  =  attT[e,n](st)      x WoT[e,f](mv)  (+ rank-1 bias MM)
"""

import os
from contextlib import ExitStack

import numpy as np

# Stage bisect: 1=setup+copyout, 2=+video/audio prep+gT, 3=+scores/exp/rs,
# 4=full (default)
KMODE = int(os.environ.get("KMODE", "4"))

import concourse.bass as bass
import concourse.mybir as mybir
import concourse.tile as tile
from concourse import bacc
from concourse.bass_utils import run_bass_kernel_spmd
from concourse.masks import make_identity

B, SEQ, D = 16, 2048, 512
NCORES = 8
BL = B // NCORES          # batches per core
P = 128
DC = D // P               # 4 chunks of the model dim
MC = SEQ // P             # 16 m-chunks per batch
NTW = 512                 # n-tile width
NT = SEQ // NTW           # 4 n-tiles per batch
NSC = NTW // P            # 4 n-subchunks per n-tile
SCALE = 1.0 / float(np.sqrt(D))

F32 = mybir.dt.float32
FMM = mybir.dt.float32r   # matmul operand dtype


def _body(tc, ctx, has_ba=False):
    nc = tc.nc
    audio = nc.t_audio.ap()
    video = nc.t_video.ap()
    out = nc.t_out.ap()

    const = ctx.enter_context(tc.tile_pool(name="const", bufs=1))
    ps_big = ctx.enter_context(tc.tile_pool(name="ps_big", bufs=4, space="PSUM"))
    ps_tp = ctx.enter_context(tc.tile_pool(name="ps_tp", bufs=3, space="PSUM"))
    ps_rs = ctx.enter_context(tc.tile_pool(name="ps_rs", bufs=1, space="PSUM"))

    # ---- constants ----
    ident = const.tile([P, P], F32, tag="ident")
    make_identity(nc, ident[:])
    ones_f32 = const.tile([P, P], F32, tag="ones_f32")
    nc.gpsimd.memset(ones_f32[:], 1.0)
    ones_col = const.tile([P, 1], FMM, tag="ones_col")
    nc.vector.tensor_copy(ones_col[:], ones_f32[:, 0:1])
    ones_row = const.tile([1, P], FMM, tag="ones_row")
    nc.vector.tensor_copy(ones_row[:], ones_f32[0:1, :])

    m_sb = const.tile([P, DC, D], FMM, tag="m_sb")
    woT = const.tile([P, DC, D], FMM, tag="woT")
    bo_r = const.tile([1, D], FMM, tag="bo_r")
    q_bc = const.tile([P, D], F32, tag="q_bc") if has_ba else None

    # ---- setup: weights, M = Wa^T Wv, WoT, biases, q ----
    with tc.tile_pool(name="setup", bufs=1) as setup:
        wa_sb = setup.tile([P, DC, D], F32, tag="wa_sb")
        wv_sb = setup.tile([P, DC, D], F32, tag="wv_sb")
        nc.sync.dma_start(wa_sb[:], nc.t_wa.ap().rearrange("(c p) d -> p c d", p=P))
        nc.sync.dma_start(wv_sb[:], nc.t_wv.ap().rearrange("(c p) d -> p c d", p=P))
        wa_r = setup.tile([P, DC, D], FMM, tag="wa_r")
        wv_r = setup.tile([P, DC, D], FMM, tag="wv_r")
        for ec in range(DC):
            nc.scalar.copy(wa_r[:, ec, :], wa_sb[:, ec, :])
            nc.scalar.copy(wv_r[:, ec, :], wv_sb[:, ec, :])

        for d1c in range(DC):
            pm = ps_big.tile([P, NTW], F32, tag="big")
            for ec in range(DC):
                nc.tensor.matmul(pm[:], wa_r[:, ec, d1c * P:(d1c + 1) * P],
                                 wv_r[:, ec, :], start=(ec == 0),
                                 stop=(ec == DC - 1))
            nc.scalar.copy(m_sb[:, d1c, :], pm[:])

        wo_sb = setup.tile([P, DC, D], F32, tag="wo_sb")
        nc.sync.dma_start(wo_sb[:], nc.t_wo.ap().rearrange("(c p) d -> p c d", p=P))
        for fc in range(DC):
            for ec in range(DC):
                pt = ps_tp.tile([P, P], F32, tag="tp")
                nc.tensor.transpose(pt[:], wo_sb[:, fc, ec * P:(ec + 1) * P],
                                    ident[:])
                nc.vector.tensor_copy(woT[:, ec, fc * P:(fc + 1) * P], pt[:])

        bo_sb = setup.tile([1, D], F32, tag="bo_sb")
        nc.sync.dma_start(bo_sb[:], nc.t_bo.ap().rearrange("(o d) -> o d", o=1))
        nc.vector.tensor_copy(bo_r[:], bo_sb[:])

        if not has_ba:
            ba_sb = None
        if has_ba:
            ba_sb = setup.tile([P, DC], F32, tag="ba_sb")
            nc.sync.dma_start(ba_sb[:], nc.t_ba.ap().rearrange("(c p) -> p c", p=P))
            ba_r = setup.tile([P, DC], FMM, tag="ba_r")
            nc.vector.tensor_copy(ba_r[:], ba_sb[:])

            # q = Wv^T b_a [1, d] -> broadcast to q_bc [128, d]
            pq = ps_rs.tile([1, D], F32, tag="rs")
            for ec in range(DC):
                nc.tensor.matmul(pq[:], ba_r[:, ec:ec + 1], wv_r[:, ec, :],
                                 start=(ec == 0), stop=(ec == DC - 1))
            # pre-scale q by 1/sqrt(d): w_col comes out pre-scaled for exp bias
            q_row = setup.tile([1, D], FMM, tag="q_row")
            nc.scalar.mul(q_row[:], pq[:], SCALE)
            pqb = ps_big.tile([P, D], F32, tag="big")
            nc.tensor.matmul(pqb[:], ones_row[:], q_row[:], start=True, stop=True)
            nc.vector.tensor_copy(q_bc[:], pqb[:])

    # ---- main pools ----
    vid = ctx.enter_context(tc.tile_pool(name="vid", bufs=1))
    vraw = ctx.enter_context(tc.tile_pool(name="vraw", bufs=4))
    araw = ctx.enter_context(tc.tile_pool(name="araw", bufs=4))
    nt_pool = ctx.enter_context(tc.tile_pool(name="nt", bufs=2))
    exp_pool = ctx.enter_context(tc.tile_pool(name="expp", bufs=1))
    outp = ctx.enter_context(tc.tile_pool(name="outp", bufs=4))
    small = ctx.enter_context(tc.tile_pool(name="small", bufs=2))

    if KMODE < 4:
        # bisect modes: cover the output via DRAM->DRAM copy; stage tiles
        # below are additionally DMA'd over parts of it to defeat DCE
        nc.sync.dma_start(out[:, :], audio[:, :])
    if KMODE == 1:
        return

    for b in range(BL):
        b0 = b * SEQ
        # video: load + round (att stationary) + w-vector + transpose
        video_r = vid.tile([P, MC, D], FMM, tag="video_r")
        videoT = vid.tile([P, DC, SEQ], FMM, tag="videoT")
        if has_ba:
            w_col = vid.tile([P, MC, 1], F32, tag="w_col")
            wsc = vid.tile([P, D], F32, tag="wsc")
        for mc in range(MC):
            vr = vraw.tile([P, D], F32, tag="vraw")
            nc.scalar.dma_start(vr[:], video[b0 + mc * P:b0 + (mc + 1) * P, :])
            # round-copy on ACT: DVE is the video-prep bottleneck
            nc.scalar.copy(video_r[:, mc, :], vr[:])
            if has_ba:
                # w = (video @ q)*scale, via mul+reduce (tensor_tensor_reduce
                # wedges the device -- see micro.py op E)
                nc.vector.tensor_mul(wsc[:], vr[:], q_bc[:])
                nc.vector.reduce_sum(w_col[:, mc, :], wsc[:],
                                     axis=mybir.AxisListType.X)
            for dc in range(DC):
                pt = ps_tp.tile([P, P], F32, tag="tp")
                nc.tensor.transpose(pt[:], vr[:, dc * P:(dc + 1) * P], ident[:])
                # split psum evictions across DVE/ACT
                if dc % 2 == 0:
                    nc.vector.tensor_copy(videoT[:, dc, mc * P:(mc + 1) * P], pt[:])
                else:
                    nc.scalar.copy(videoT[:, dc, mc * P:(mc + 1) * P], pt[:])

        for nt in range(NT):
            n0 = b0 + nt * NTW
            # audio slice -> aT -> gT
            aT = nt_pool.tile([P, DC, NTW], FMM, tag="aT")
            for rc in range(NSC):
                ar = araw.tile([P, D], F32, tag="araw")
                nc.scalar.dma_start(ar[:], audio[n0 + rc * P:n0 + (rc + 1) * P, :])
                for dc in range(DC):
                    pt = ps_tp.tile([P, P], F32, tag="tp")
                    nc.tensor.transpose(pt[:], ar[:, dc * P:(dc + 1) * P], ident[:])
                    nc.vector.tensor_copy(aT[:, dc, rc * P:(rc + 1) * P], pt[:])
            gT = nt_pool.tile([P, DC, NTW], FMM, tag="gT")
            for d2c in range(DC):
                pg = ps_big.tile([P, NTW], F32, tag="big")
                for d1c in range(DC):
                    nc.tensor.matmul(pg[:], m_sb[:, d1c, d2c * P:(d2c + 1) * P],
                                     aT[:, d1c, :],
                                     start=(d1c == 0), stop=(d1c == DC - 1))
                nc.scalar.copy(gT[:, d2c, :], pg[:])

            if KMODE == 2:
                nc.sync.dma_start(out[n0:n0 + P, :], gT[:, 0, :].bitcast(F32))
                continue

            # scores -> exp -> rowsums
            exp_t = exp_pool.tile([P, MC, NTW], FMM, tag="exp_t")
            prs = ps_rs.tile([1, NTW], F32, tag="rs")
            for mc in range(MC):
                psc = ps_big.tile([P, NTW], F32, tag="big")
                for d2c in range(DC):
                    nc.tensor.matmul(psc[:], videoT[:, d2c, mc * P:(mc + 1) * P],
                                     gT[:, d2c, :],
                                     start=(d2c == 0), stop=(d2c == DC - 1))
                nc.scalar.activation(exp_t[:, mc, :], psc[:],
                                     mybir.ActivationFunctionType.Exp,
                                     bias=(w_col[:, mc, :] if has_ba else 0.0),
                                     scale=SCALE)
                nc.tensor.matmul(prs[:], ones_col[:], exp_t[:, mc, :],
                                 start=(mc == 0), stop=(mc == MC - 1))

            # denominators: round, reciprocal, column-ize via K=1 matmul
            rs_row = small.tile([1, NTW], F32, tag="rs_row")
            nc.scalar.copy(rs_row[:], prs[:])
            rs_row_r = small.tile([1, NTW], FMM, tag="rs_row_r")
            nc.vector.tensor_copy(rs_row_r[:], rs_row[:])
            rr_row = small.tile([1, NTW], F32, tag="rr_row")
            nc.vector.reciprocal(rr_row[:], rs_row[:])
            recip_col = small.tile([P, NSC, 1], F32, tag="recip_col")
            for ns in range(NSC):
                # K=1 fp32 matmul (fp32r disallows odd free counts here)
                prc = ps_tp.tile([P, 1], F32, tag="tp")
                nc.tensor.matmul(prc[:], rr_row[:, ns * P:(ns + 1) * P],
                                 ones_f32[0:1, 0:1], start=True, stop=True)
                nc.vector.tensor_copy(recip_col[:, ns, :], prc[:])

            if KMODE == 3:
                nc.sync.dma_start(out[n0:n0 + P, :], exp_t[:, 0, :].bitcast(F32))
                nc.sync.dma_start(out[n0 + P:n0 + P + 1, 0:NSC],
                                  recip_col[0:1, :, 0])
                continue

            # att_T
            att_sb = nt_pool.tile([P, DC, NTW], FMM, tag="att_sb")
            for dc in range(DC):
                pa = ps_big.tile([P, NTW], F32, tag="big")
                for mc in range(MC):
                    nc.tensor.matmul(pa[:], video_r[:, mc, dc * P:(dc + 1) * P],
                                     exp_t[:, mc, :],
                                     start=(mc == 0), stop=(mc == MC - 1))
                nc.scalar.copy(att_sb[:, dc, :], pa[:])

            # out projection + rank-1 bias + normalize
            for ns in range(NSC):
                po = ps_big.tile([P, D], F32, tag="big")
                for ec in range(DC):
                    nc.tensor.matmul(po[:], att_sb[:, ec, ns * P:(ns + 1) * P],
                                     woT[:, ec, :], start=(ec == 0), stop=False)
                nc.tensor.matmul(po[:], rs_row_r[:, ns * P:(ns + 1) * P], bo_r[:],
                                 start=False, stop=True)
                o_sb = outp.tile([P, D], F32, tag="o_sb")
                nc.scalar.activation(o_sb[:], po[:],
                                     mybir.ActivationFunctionType.Copy,
                                     scale=recip_col[:, ns, :])
                nc.sync.dma_start(out[n0 + ns * P:n0 + (ns + 1) * P, :], o_sb[:])


_NC_CACHE = {}


def _build(has_ba=False):
    if has_ba in _NC_CACHE:
        return _NC_CACHE[has_ba]
    nc = bacc.Bacc("TRN2", target_bir_lowering=False, debug=False,
                   num_devices=NCORES)
    nc.t_audio = nc.dram_tensor("audio", [BL * SEQ, D], F32, kind="ExternalInput")
    nc.t_video = nc.dram_tensor("video", [BL * SEQ, D], F32, kind="ExternalInput")
    nc.t_wa = nc.dram_tensor("w_a", [D, D], F32, kind="ExternalInput")
    nc.t_wv = nc.dram_tensor("w_v", [D, D], F32, kind="ExternalInput")
    nc.t_wo = nc.dram_tensor("w_o", [D, D], F32, kind="ExternalInput")
    nc.t_ba = nc.dram_tensor("b_a", [D], F32, kind="ExternalInput")
    nc.t_bo = nc.dram_tensor("b_o", [D], F32, kind="ExternalInput")
    nc.t_out = nc.dram_tensor("out", [BL * SEQ, D], F32, kind="ExternalOutput")
    with tile.TileContext(nc) as tc:
        with ExitStack() as ctx:
            _body(tc, ctx, has_ba=has_ba)
    nc.compile()
    _NC_CACHE[has_ba] = nc
    return nc


def kernel(audio, video, W_a, b_a, W_v, b_v, W_o, b_o, _trace=False):
    nc = _build(has_ba=bool(np.any(np.asarray(b_a))))
    audio = np.ascontiguousarray(audio, dtype=np.float32)
    video = np.ascontiguousarray(video, dtype=np.float32)
    shared = {
        "w_a": np.ascontiguousarray(W_a, dtype=np.float32),
        "w_v": np.ascontiguousarray(W_v, dtype=np.float32),
        "w_o": np.ascontiguousarray(W_o, dtype=np.float32),
        "b_a": np.ascontiguousarray(b_a, dtype=np.float32),
        "b_o": np.ascontiguousarray(b_o, dtype=np.float32),
    }
    in_maps = []
    for c in range(NCORES):
        sl = slice(c * BL, (c + 1) * BL)
        in_maps.append({
            "audio": audio[sl].reshape(BL * SEQ, D),
            "video": video[sl].reshape(BL * SEQ, D),
            **shared,
        })
    res = run_bass_kernel_spmd(nc, in_maps, core_ids=list(range(NCORES)),
                               trace=_trace)
    out = np.concatenate(
        [res.results[c]["out"].reshape(BL, SEQ, D) for c in range(NCORES)],
        axis=0)
    if _trace:
        kernel.last_exec_time_ns = res.exec_time_ns
        kernel.last_results = res
    return out



# revision 2
# speedup vs baseline: 1.1125x; 1.1125x over previous
"""CrossModalAttention kernel for 8 Trainium2 NeuronCores.

Data-parallel over batch: B=16 -> 2 batches per core.

Math (per batch, with A=audio [N,d], B=video [M,d]):
  scores*sqrt(d) = (A Wa^T + 1 b_a^T)(B Wv^T + 1 b_v^T)^T
                 = A M B^T + (row-constant terms) + 1_n w^T
  with M = Wa^T Wv, w = B (Wv^T b_a).  Row-constant terms drop inside
  softmax, and max-subtraction is skipped: scores are ~N(0,1), far from
  fp32 exp overflow.
  attn     = exp_s / rowsum, with exp_s kept transposed [m, n]
  att_T[d,n] = sum_m video[m,d] exp_s[m,n]
  out[n,f] = (att_T^T @ Wo^T) * (1/rowsum[n]) + b_o

All matmul operands are bf16 (1 cyc/row on PE, fp32 PSUM accumulation);
softmax internals stay fp32.  K is always on partitions.  All transposes
run on the DMA XBAR (2-byte dtype): one dma transpose per [128,512] bf16
tile lands it in chunk-major transposed layout, so the PE does zero
transpose work:

  aT[d1c,n]  <- DMA-transpose of bf16 audio slice
  gT[d2,n]  =  M[d1,d2](st)       x aT(mv)
  sT[m,n]   =  videoT[d2,m](st)   x gT(mv);   exp on ACT (bf16 out)
  acc[p,n]  += exp[p + 128*mc, n]             (DVE partial rowsum)
  rs[n,1]   =  acc[p,nslice](st)  x ones[p,1](mv)   (N=1 matmul)
  attT[d,n] =  video_r[m,d](st)   x exp(mv)
  out[n,f]  =  attT[e,n](st)      x WoT[e,f](mv); *recip on ACT evict
"""

import os
from contextlib import ExitStack

import numpy as np

# Stage bisect: 1=setup+copyout, 2=+audio prep+gT, 3=+scores/exp/rs,
# 4=full (default)
KMODE = int(os.environ.get("KMODE", "4"))

import concourse.bass as bass
import concourse.mybir as mybir
import concourse.tile as tile
from concourse import bacc
from concourse.bass_utils import run_bass_kernel_spmd

B, SEQ, D = 16, 2048, 512
NCORES = 8
BL = B // NCORES          # batches per core
P = 128
DC = D // P               # 4 chunks of the model dim
MC = SEQ // P             # 16 m-chunks per batch
NTW = 512                 # n-tile width
NT = SEQ // NTW           # 4 n-tiles per batch
NSC = NTW // P            # 4 n-subchunks per n-tile
SCALE = 1.0 / float(np.sqrt(D))

F32 = mybir.dt.float32
BF16 = mybir.dt.bfloat16
FR = mybir.dt.float32r


def _body(tc, ctx, has_ba=False):
    nc = tc.nc
    audio = nc.t_audio.ap()
    video = nc.t_video.ap()
    out = nc.t_out.ap()

    const = ctx.enter_context(tc.tile_pool(name="const", bufs=1))
    ps_big = ctx.enter_context(tc.tile_pool(name="ps_big", bufs=6, space="PSUM"))
    ps_rs = ctx.enter_context(tc.tile_pool(name="ps_rs", bufs=2, space="PSUM"))

    # ---- constants ----
    ones_f32 = const.tile([P, P], F32, tag="ones_f32")
    nc.gpsimd.memset(ones_f32[:], 1.0)
    ones_col = const.tile([P, 1], BF16, tag="ones_col")
    nc.vector.tensor_copy(ones_col[:], ones_f32[:, 0:1])
    ones_row = const.tile([1, P], BF16, tag="ones_row")
    nc.vector.tensor_copy(ones_row[:], ones_f32[0:1, :])

    m_sb = const.tile([P, DC, D], BF16, tag="m_sb")
    woT = const.tile([P, DC, D], BF16, tag="woT")
    bo_bc = const.tile([P, D], F32, tag="bo_bc")
    q_bc = const.tile([P, D], F32, tag="q_bc") if has_ba else None

    # ---- setup: M = Wa^T Wv (fp32r), WoT (dma transpose), bias bcasts ----
    with tc.tile_pool(name="setup", bufs=1) as setup:
        wa_sb = setup.tile([P, DC, D], F32, tag="wa_sb")
        wv_sb = setup.tile([P, DC, D], F32, tag="wv_sb")
        nc.sync.dma_start(wa_sb[:], nc.t_wa.ap().rearrange("(c p) d -> p c d", p=P))
        nc.sync.dma_start(wv_sb[:], nc.t_wv.ap().rearrange("(c p) d -> p c d", p=P))
        wa_r = setup.tile([P, DC, D], FR, tag="wa_r")
        wv_r = setup.tile([P, DC, D], FR, tag="wv_r")
        for ec in range(DC):
            nc.scalar.copy(wa_r[:, ec, :], wa_sb[:, ec, :])
            nc.scalar.copy(wv_r[:, ec, :], wv_sb[:, ec, :])

        for d1c in range(DC):
            pm = ps_big.tile([P, NTW], F32, tag="big")
            for ec in range(DC):
                nc.tensor.matmul(pm[:], wa_r[:, ec, d1c * P:(d1c + 1) * P],
                                 wv_r[:, ec, :], start=(ec == 0),
                                 stop=(ec == DC - 1))
            nc.scalar.copy(m_sb[:, d1c, :], pm[:])

        wo_sb = setup.tile([P, DC, D], F32, tag="wo_sb")
        nc.sync.dma_start(wo_sb[:], nc.t_wo.ap().rearrange("(c p) d -> p c d", p=P))
        wo_bf = setup.tile([P, DC, D], BF16, tag="wo_bf")
        nc.vector.tensor_copy(wo_bf[:], wo_sb[:])
        # woT[e%128, ec, f] = Wo[f, e]: one XBAR transpose per f-chunk
        for fc in range(DC):
            nc.sync.dma_start(woT[:, :, fc * P:(fc + 1) * P], wo_bf[:, fc, :],
                              transpose=True)

        bo_sb = setup.tile([1, D], F32, tag="bo_sb")
        nc.sync.dma_start(bo_sb[:], nc.t_bo.ap().rearrange("(o d) -> o d", o=1))
        bo_bf = setup.tile([1, D], BF16, tag="bo_bf")
        nc.vector.tensor_copy(bo_bf[:], bo_sb[:])
        pb = ps_big.tile([P, NTW], F32, tag="big")
        nc.tensor.matmul(pb[:], ones_row[:], bo_bf[:], start=True, stop=True)
        nc.vector.tensor_copy(bo_bc[:], pb[:])

        if has_ba:
            ba_sb = setup.tile([P, DC], F32, tag="ba_sb")
            nc.sync.dma_start(ba_sb[:], nc.t_ba.ap().rearrange("(c p) -> p c", p=P))
            ba_r = setup.tile([P, DC], FR, tag="ba_r")
            nc.vector.tensor_copy(ba_r[:], ba_sb[:])
            # q = Wv^T b_a [1, d], pre-scaled by 1/sqrt(d); bcast to [128, d]
            pq = ps_big.tile([P, NTW], F32, tag="big")
            for ec in range(DC):
                nc.tensor.matmul(pq[0:1, :], ba_r[:, ec:ec + 1], wv_r[:, ec, :],
                                 start=(ec == 0), stop=(ec == DC - 1))
            q_row = setup.tile([1, D], BF16, tag="q_row")
            nc.scalar.mul(q_row[:], pq[0:1, :], SCALE)
            pqb = ps_big.tile([P, NTW], F32, tag="big")
            nc.tensor.matmul(pqb[:], ones_row[:], q_row[:], start=True, stop=True)
            nc.vector.tensor_copy(q_bc[:], pqb[:])

    # ---- main pools ----
    vid = ctx.enter_context(tc.tile_pool(name="vid", bufs=2))
    vraw = ctx.enter_context(tc.tile_pool(name="vraw", bufs=4))
    araw = ctx.enter_context(tc.tile_pool(name="araw", bufs=4))
    acast = ctx.enter_context(tc.tile_pool(name="acast", bufs=4))
    nt_pool = ctx.enter_context(tc.tile_pool(name="nt", bufs=2))
    exp_pool = ctx.enter_context(tc.tile_pool(name="expp", bufs=2))
    accp = ctx.enter_context(tc.tile_pool(name="accp", bufs=2))
    outp = ctx.enter_context(tc.tile_pool(name="outp", bufs=4))
    small = ctx.enter_context(tc.tile_pool(name="small", bufs=2))

    if KMODE < 4:
        # bisect modes: cover the output via DRAM->DRAM copy; stage tiles
        # below are additionally DMA'd over parts of it to defeat DCE
        nc.sync.dma_start(out[:, :], audio[:, :])
    if KMODE == 1:
        return

    for b in range(BL):
        b0 = b * SEQ
        # video: load, cast to bf16 (attn values, stationary), XBAR-transpose
        video_r = vid.tile([P, MC, D], BF16, tag="video_r")
        videoT = vid.tile([P, MC, DC, P], BF16, tag="videoT")
        if has_ba:
            w_col = vid.tile([P, MC, 1], F32, tag="w_col")
            wsc = vid.tile([P, D], F32, tag="wsc")
        for mc in range(MC):
            vr = vraw.tile([P, D], F32, tag="vraw")
            nc.scalar.dma_start(vr[:], video[b0 + mc * P:b0 + (mc + 1) * P, :])
            nc.vector.tensor_copy(video_r[:, mc, :], vr[:])
            nc.sync.dma_start(videoT[:, mc, :, :], video_r[:, mc, :],
                              transpose=True)
            if has_ba:
                # w = (video @ q)*scale, via mul+reduce (tensor_tensor_reduce
                # wedges the device)
                nc.vector.tensor_mul(wsc[:], vr[:], q_bc[:])
                nc.vector.reduce_sum(w_col[:, mc, :], wsc[:],
                                     axis=mybir.AxisListType.X)

        for nt in range(NT):
            n0 = b0 + nt * NTW
            # audio slice -> bf16 -> aT via XBAR
            aT = nt_pool.tile([P, DC, NSC, P], BF16, tag="aT")
            for rc in range(NSC):
                ar = araw.tile([P, D], F32, tag="araw")
                nc.scalar.dma_start(ar[:], audio[n0 + rc * P:n0 + (rc + 1) * P, :])
                ab = acast.tile([P, D], BF16, tag="acast")
                nc.vector.tensor_copy(ab[:], ar[:])
                nc.sync.dma_start(aT[:, :, rc, :], ab[:], transpose=True)
            gT = nt_pool.tile([P, DC, NTW], BF16, tag="gT")
            for d2c in range(DC):
                pg = ps_big.tile([P, NTW], F32, tag="big")
                for d1c in range(DC):
                    nc.tensor.matmul(pg[:], m_sb[:, d1c, d2c * P:(d2c + 1) * P],
                                     aT[:, d1c, :, :],
                                     start=(d1c == 0), stop=(d1c == DC - 1))
                nc.scalar.copy(gT[:, d2c, :], pg[:])

            if KMODE == 2:
                nc.sync.dma_start(out[n0:n0 + P, 0:NTW // 2],
                                  gT[:, 0, :].bitcast(F32))
                continue

            # scores -> exp (bf16) -> partial rowsums on DVE
            exp_t = exp_pool.tile([P, MC, NTW], BF16, tag="exp_t")
            acc = accp.tile([P, NTW], BF16, tag="acc")
            for mc in range(MC):
                psc = ps_big.tile([P, NTW], F32, tag="big")
                for d2c in range(DC):
                    nc.tensor.matmul(psc[:], videoT[:, mc, d2c, :],
                                     gT[:, d2c, :],
                                     start=(d2c == 0), stop=(d2c == DC - 1))
                nc.scalar.activation(exp_t[:, mc, :], psc[:],
                                     mybir.ActivationFunctionType.Exp,
                                     bias=(w_col[:, mc, :] if has_ba else 0.0),
                                     scale=SCALE)
                if mc == 0:
                    nc.vector.tensor_copy(acc[:], exp_t[:, mc, :])
                else:
                    nc.vector.tensor_add(acc[:], acc[:], exp_t[:, mc, :])

            # rowsum columns: N=1 matmul per n-subchunk, then reciprocal
            recip_col = small.tile([P, NSC, 1], F32, tag="recip_col")
            for ns in range(NSC):
                prc = ps_rs.tile([P, 1], F32, tag="rs")
                nc.tensor.matmul(prc[:], acc[:, ns * P:(ns + 1) * P],
                                 ones_col[:], start=True, stop=True)
                nc.vector.reciprocal(recip_col[:, ns, :], prc[:])

            if KMODE == 3:
                nc.sync.dma_start(out[n0:n0 + P, 0:NTW // 2],
                                  exp_t[:, 0, :].bitcast(F32))
                nc.sync.dma_start(out[n0 + P:n0 + P + 1, 0:NSC],
                                  recip_col[0:1, :, 0])
                continue

            # att_T
            att_sb = nt_pool.tile([P, DC, NTW], BF16, tag="att_sb")
            for dc in range(DC):
                pa = ps_big.tile([P, NTW], F32, tag="big")
                for mc in range(MC):
                    nc.tensor.matmul(pa[:], video_r[:, mc, dc * P:(dc + 1) * P],
                                     exp_t[:, mc, :],
                                     start=(mc == 0), stop=(mc == MC - 1))
                nc.vector.tensor_copy(att_sb[:, dc, :], pa[:])

            # out projection, normalize on ACT evict, bias on DVE
            for ns in range(NSC):
                po = ps_big.tile([P, D], F32, tag="big")
                for ec in range(DC):
                    nc.tensor.matmul(po[:], att_sb[:, ec, ns * P:(ns + 1) * P],
                                     woT[:, ec, :], start=(ec == 0),
                                     stop=(ec == DC - 1))
                o_sb = outp.tile([P, D], F32, tag="o_sb")
                nc.scalar.activation(o_sb[:], po[:],
                                     mybir.ActivationFunctionType.Copy,
                                     scale=recip_col[:, ns, :])
                nc.vector.tensor_add(o_sb[:], o_sb[:], bo_bc[:])
                nc.sync.dma_start(out[n0 + ns * P:n0 + (ns + 1) * P, :], o_sb[:])


_NC_CACHE = {}


def _build(has_ba=False):
    if has_ba in _NC_CACHE:
        return _NC_CACHE[has_ba]
    nc = bacc.Bacc("TRN2", target_bir_lowering=False, debug=False,
                   num_devices=NCORES)
    nc.t_audio = nc.dram_tensor("audio", [BL * SEQ, D], F32, kind="ExternalInput")
    nc.t_video = nc.dram_tensor("video", [BL * SEQ, D], F32, kind="ExternalInput")
    nc.t_wa = nc.dram_tensor("w_a", [D, D], F32, kind="ExternalInput")
    nc.t_wv = nc.dram_tensor("w_v", [D, D], F32, kind="ExternalInput")
    nc.t_wo = nc.dram_tensor("w_o", [D, D], F32, kind="ExternalInput")
    nc.t_ba = nc.dram_tensor("b_a", [D], F32, kind="ExternalInput")
    nc.t_bo = nc.dram_tensor("b_o", [D], F32, kind="ExternalInput")
    nc.t_out = nc.dram_tensor("out", [BL * SEQ, D], F32, kind="ExternalOutput")
    with tile.TileContext(nc) as tc:
        with ExitStack() as ctx:
            _body(tc, ctx, has_ba=has_ba)
    nc.compile()
    _NC_CACHE[has_ba] = nc
    return nc


def kernel(audio, video, W_a, b_a, W_v, b_v, W_o, b_o, _trace=False):
    nc = _build(has_ba=bool(np.any(np.asarray(b_a))))
    audio = np.ascontiguousarray(audio, dtype=np.float32)
    video = np.ascontiguousarray(video, dtype=np.float32)
    shared = {
        "w_a": np.ascontiguousarray(W_a, dtype=np.float32),
        "w_v": np.ascontiguousarray(W_v, dtype=np.float32),
        "w_o": np.ascontiguousarray(W_o, dtype=np.float32),
        "b_a": np.ascontiguousarray(b_a, dtype=np.float32),
        "b_o": np.ascontiguousarray(b_o, dtype=np.float32),
    }
    in_maps = []
    for c in range(NCORES):
        sl = slice(c * BL, (c + 1) * BL)
        in_maps.append({
            "audio": audio[sl].reshape(BL * SEQ, D),
            "video": video[sl].reshape(BL * SEQ, D),
            **shared,
        })
    res = run_bass_kernel_spmd(nc, in_maps, core_ids=list(range(NCORES)),
                               trace=_trace)
    out = np.concatenate(
        [res.results[c]["out"].reshape(BL, SEQ, D) for c in range(NCORES)],
        axis=0)
    if _trace:
        kernel.last_exec_time_ns = res.exec_time_ns
        kernel.last_results = res
    return out


# revision 7
# speedup vs baseline: 1.3932x; 1.2523x over previous
"""CrossModalAttention kernel for 8 Trainium2 NeuronCores.

Data-parallel over batch: B=16 -> 2 batches per core.

Math (per batch, with A=audio [N,d], B=video [M,d]):
  scores*sqrt(d) = (A Wa^T + 1 b_a^T)(B Wv^T + 1 b_v^T)^T
                 = A M B^T + (row-constant terms) + 1_n w^T
  with M = Wa^T Wv, w = B (Wv^T b_a).  Row-constant terms drop inside
  softmax, and max-subtraction is skipped: scores are ~N(0,1), far from
  fp32 exp overflow.
  attn     = exp_s / rowsum, with exp_s kept transposed [m, n]
  att_T[d,n] = sum_m video[m,d] exp_s[m,n]
  out[n,f] = (att_T^T @ Wo^T) * (1/rowsum[n]) + b_o

All matmul operands are bf16 (1 cyc/row on PE, fp32 PSUM accumulation);
softmax internals stay fp32.  K is always on partitions.  All transposes
run on the DMA XBAR (2-byte dtype): one dma transpose per [128,512] bf16
tile lands it in chunk-major transposed layout, so the PE does zero
transpose work:

  aT[d1c,n]  <- DMA-transpose of bf16 audio slice
  gT[d2,n]  =  M[d1,d2](st)       x aT(mv)
  sT[m,n]   =  videoT[d2,m](st)   x gT(mv);   exp on ACT (bf16 out)
  acc[p,n]  += exp[p + 128*mc, n]             (DVE partial rowsum)
  rs[n,1]   =  acc[p,nslice](st)  x ones[p,1](mv)   (N=1 matmul)
  attT[d,n] =  video_r[m,d](st)   x exp(mv)
  out[n,f]  =  attT[e,n](st)      x WoT[e,f](mv); *recip on ACT evict
"""

import os
from contextlib import ExitStack

import numpy as np

# Stage bisect: 1=setup+copyout, 2=+audio prep+gT, 3=+scores/exp/rs,
# 4=full (default)
KMODE = int(os.environ.get("KMODE", "4"))

import concourse.bass as bass
import concourse.mybir as mybir
import concourse.tile as tile
from concourse import bacc
from concourse.bass_utils import run_bass_kernel_spmd

B, SEQ, D = 16, 2048, 512
NCORES = 8
BL = B // NCORES          # batches per core
P = 128
DC = D // P               # 4 chunks of the model dim
MC = SEQ // P             # 16 m-chunks per batch
NTW = 512                 # n-tile width
NT = SEQ // NTW           # 4 n-tiles per batch
NSC = NTW // P            # 4 n-subchunks per n-tile
SCALE = 1.0 / float(np.sqrt(D))

F32 = mybir.dt.float32
BF16 = mybir.dt.bfloat16
FR = mybir.dt.float32r


def _body(tc, ctx, has_ba=False):
    nc = tc.nc
    audio = nc.t_audio.ap()
    video = nc.t_video.ap()
    out = nc.t_out.ap()

    const = ctx.enter_context(tc.tile_pool(name="const", bufs=1))
    ps_big = ctx.enter_context(tc.tile_pool(name="ps_big", bufs=6, space="PSUM"))
    ps_rs = ctx.enter_context(tc.tile_pool(name="ps_rs", bufs=2, space="PSUM"))

    # ---- constants ----
    ones_f32 = const.tile([P, P], F32, tag="ones_f32")
    nc.gpsimd.memset(ones_f32[:], 1.0)
    ones_col = const.tile([P, 1], BF16, tag="ones_col")
    nc.vector.tensor_copy(ones_col[:], ones_f32[:, 0:1])
    ones_row = const.tile([1, P], BF16, tag="ones_row")
    nc.vector.tensor_copy(ones_row[:], ones_f32[0:1, :])

    m_sb = const.tile([P, DC, D], BF16, tag="m_sb")
    woT = const.tile([P, DC, DC, P], BF16, tag="woT")
    bo_bc = const.tile([P, D], F32, tag="bo_bc")
    q_bc = const.tile([P, D], F32, tag="q_bc") if has_ba else None

    # ---- setup: M = Wa^T Wv (fp32r), WoT (dma transpose), bias bcasts ----
    with tc.tile_pool(name="setup", bufs=1) as setup:
        wa_sb = setup.tile([P, DC, D], F32, tag="wa_sb")
        wv_sb = setup.tile([P, DC, D], F32, tag="wv_sb")
        nc.sync.dma_start(wa_sb[:], nc.t_wa.ap().rearrange("(c p) d -> p c d", p=P))
        nc.sync.dma_start(wv_sb[:], nc.t_wv.ap().rearrange("(c p) d -> p c d", p=P))
        wa_r = setup.tile([P, DC, D], FR, tag="wa_r")
        wv_r = setup.tile([P, DC, D], FR, tag="wv_r")
        for ec in range(DC):
            nc.scalar.copy(wa_r[:, ec, :], wa_sb[:, ec, :])
            nc.scalar.copy(wv_r[:, ec, :], wv_sb[:, ec, :])

        for d1c in range(DC):
            pm = ps_big.tile([P, NTW], F32, tag="big")
            for ec in range(DC):
                nc.tensor.matmul(pm[:], wa_r[:, ec, d1c * P:(d1c + 1) * P],
                                 wv_r[:, ec, :], start=(ec == 0),
                                 stop=(ec == DC - 1))
            nc.scalar.copy(m_sb[:, d1c, :], pm[:])

        wo_sb = setup.tile([P, DC, D], F32, tag="wo_sb")
        nc.sync.dma_start(wo_sb[:], nc.t_wo.ap().rearrange("(c p) d -> p c d", p=P))
        wo_bf = setup.tile([P, DC, D], BF16, tag="wo_bf")
        nc.vector.tensor_copy(wo_bf[:], wo_sb[:])
        # woT[e%128, fc, ec, f%128] = Wo[fc*128+f', ec*128+e']: ONE XBAR
        # transpose of the [128, 2048] bf16 tile (c = fc*4+ec chunk-major)
        nc.sync.dma_start(woT[:], wo_bf[:], transpose=True)

        bo_sb = setup.tile([1, D], F32, tag="bo_sb")
        nc.sync.dma_start(bo_sb[:], nc.t_bo.ap().rearrange("(o d) -> o d", o=1))
        bo_bf = setup.tile([1, D], BF16, tag="bo_bf")
        nc.vector.tensor_copy(bo_bf[:], bo_sb[:])
        pb = ps_big.tile([P, NTW], F32, tag="big")
        nc.tensor.matmul(pb[:], ones_row[:], bo_bf[:], start=True, stop=True)
        nc.vector.tensor_copy(bo_bc[:], pb[:])

        if has_ba:
            ba_sb = setup.tile([P, DC], F32, tag="ba_sb")
            nc.sync.dma_start(ba_sb[:], nc.t_ba.ap().rearrange("(c p) -> p c", p=P))
            ba_r = setup.tile([P, DC], FR, tag="ba_r")
            nc.vector.tensor_copy(ba_r[:], ba_sb[:])
            # q = Wv^T b_a [1, d], pre-scaled by 1/sqrt(d); bcast to [128, d]
            pq = ps_big.tile([P, NTW], F32, tag="big")
            for ec in range(DC):
                nc.tensor.matmul(pq[0:1, :], ba_r[:, ec:ec + 1], wv_r[:, ec, :],
                                 start=(ec == 0), stop=(ec == DC - 1))
            q_row = setup.tile([1, D], BF16, tag="q_row")
            nc.scalar.mul(q_row[:], pq[0:1, :], SCALE)
            pqb = ps_big.tile([P, NTW], F32, tag="big")
            nc.tensor.matmul(pqb[:], ones_row[:], q_row[:], start=True, stop=True)
            nc.vector.tensor_copy(q_bc[:], pqb[:])

    # ---- main pools ----
    vid = ctx.enter_context(tc.tile_pool(name="vid", bufs=2))
    vraw = ctx.enter_context(tc.tile_pool(name="vraw", bufs=4))
    araw = ctx.enter_context(tc.tile_pool(name="araw", bufs=4))
    acast = ctx.enter_context(tc.tile_pool(name="acast", bufs=4))
    nt_pool = ctx.enter_context(tc.tile_pool(name="nt", bufs=2))
    exp_pool = ctx.enter_context(tc.tile_pool(name="expp", bufs=2))
    accp = ctx.enter_context(tc.tile_pool(name="accp", bufs=2))
    outp = ctx.enter_context(tc.tile_pool(name="outp", bufs=4))
    small = ctx.enter_context(tc.tile_pool(name="small", bufs=2))

    if KMODE < 4:
        # bisect modes: cover the output via DRAM->DRAM copy; stage tiles
        # below are additionally DMA'd over parts of it to defeat DCE
        nc.sync.dma_start(out[:, :], audio[:, :])
    if KMODE == 1:
        return

    for b in range(BL):
        b0 = b * SEQ
        # video: load, cast to bf16 (attn values, stationary), XBAR-transpose
        video_r = vid.tile([P, MC, D], BF16, tag="video_r")
        videoT = vid.tile([P, MC, DC, P], BF16, tag="videoT")
        if has_ba:
            w_col = vid.tile([P, MC, 1], F32, tag="w_col")
            wsc = vid.tile([P, D], F32, tag="wsc")
        for mc in range(MC):
            vr = vraw.tile([P, D], F32, tag="vraw")
            nc.scalar.dma_start(vr[:], video[b0 + mc * P:b0 + (mc + 1) * P, :])
            nc.vector.tensor_copy(video_r[:, mc, :], vr[:])
            if has_ba:
                # w = (video @ q)*scale, via mul+reduce (tensor_tensor_reduce
                # wedges the device)
                nc.vector.tensor_mul(wsc[:], vr[:], q_bc[:])
                nc.vector.reduce_sum(w_col[:, mc, :], wsc[:],
                                     axis=mybir.AxisListType.X)
        # ONE XBAR transpose for the whole batch: [128, 16*512] bf16 ->
        # videoT[d2%128, mc, d2c, m%128] (c = mc*4+d2c chunk-major)
        nc.sync.dma_start(videoT[:], video_r[:], transpose=True)

        for nt in range(NT):
            n0 = b0 + nt * NTW
            # audio slice -> bf16 (contiguous [128, 2048]) -> aT via ONE XBAR
            # transpose: aT[d1%128, rc, d1c, n%128] (c = rc*4+d1c)
            ab = acast.tile([P, NSC, D], BF16, tag="acast")
            for rc in range(NSC):
                ar = araw.tile([P, D], F32, tag="araw")
                nc.scalar.dma_start(ar[:], audio[n0 + rc * P:n0 + (rc + 1) * P, :])
                nc.vector.tensor_copy(ab[:, rc, :], ar[:])
            aT = nt_pool.tile([P, NSC, DC, P], BF16, tag="aT")
            nc.sync.dma_start(aT[:], ab[:], transpose=True)
            gT = nt_pool.tile([P, DC, NTW], BF16, tag="gT")
            for d2c in range(DC):
                pg = ps_big.tile([P, NTW], F32, tag="big")
                for d1c in range(DC):
                    nc.tensor.matmul(pg[:], m_sb[:, d1c, d2c * P:(d2c + 1) * P],
                                     aT[:, :, d1c, :],
                                     start=(d1c == 0), stop=(d1c == DC - 1))
                nc.scalar.copy(gT[:, d2c, :], pg[:])

            if KMODE == 2:
                nc.sync.dma_start(out[n0:n0 + P, 0:NTW // 2],
                                  gT[:, 0, :].bitcast(F32))
                continue

            # scores -> exp (bf16) -> partial rowsums on DVE
            exp_t = exp_pool.tile([P, MC, NTW], BF16, tag="exp_t")
            acc = accp.tile([P, NTW], BF16, tag="acc")
            for mc in range(MC):
                psc = ps_big.tile([P, NTW], F32, tag="big")
                for d2c in range(DC):
                    nc.tensor.matmul(psc[:], videoT[:, mc, d2c, :],
                                     gT[:, d2c, :],
                                     start=(d2c == 0), stop=(d2c == DC - 1))
                nc.scalar.activation(exp_t[:, mc, :], psc[:],
                                     mybir.ActivationFunctionType.Exp,
                                     bias=(w_col[:, mc, :] if has_ba else 0.0),
                                     scale=SCALE)
                if mc == 0:
                    nc.vector.tensor_copy(acc[:], exp_t[:, mc, :])
                else:
                    nc.vector.tensor_add(acc[:], acc[:], exp_t[:, mc, :])

            # rowsum columns: N=1 matmul per n-subchunk, then reciprocal
            recip_col = small.tile([P, NSC, 1], F32, tag="recip_col")
            for ns in range(NSC):
                prc = ps_rs.tile([P, 1], F32, tag="rs")
                nc.tensor.matmul(prc[:], acc[:, ns * P:(ns + 1) * P],
                                 ones_col[:], start=True, stop=True)
                nc.vector.reciprocal(recip_col[:, ns, :], prc[:])

            if KMODE == 3:
                nc.sync.dma_start(out[n0:n0 + P, 0:NTW // 2],
                                  exp_t[:, 0, :].bitcast(F32))
                nc.sync.dma_start(out[n0 + P:n0 + P + 1, 0:NSC],
                                  recip_col[0:1, :, 0])
                continue

            # att_T
            att_sb = nt_pool.tile([P, DC, NTW], BF16, tag="att_sb")
            for dc in range(DC):
                pa = ps_big.tile([P, NTW], F32, tag="big")
                for mc in range(MC):
                    nc.tensor.matmul(pa[:], video_r[:, mc, dc * P:(dc + 1) * P],
                                     exp_t[:, mc, :],
                                     start=(mc == 0), stop=(mc == MC - 1))
                nc.vector.tensor_copy(att_sb[:, dc, :], pa[:])

            # out projection, normalize on ACT evict, bias on DVE
            for ns in range(NSC):
                po = ps_big.tile([P, D], F32, tag="big")
                for ec in range(DC):
                    nc.tensor.matmul(po[:], att_sb[:, ec, ns * P:(ns + 1) * P],
                                     woT[:, :, ec, :], start=(ec == 0),
                                     stop=(ec == DC - 1))
                o_sb = outp.tile([P, D], F32, tag="o_sb")
                nc.scalar.activation(o_sb[:], po[:],
                                     mybir.ActivationFunctionType.Copy,
                                     scale=recip_col[:, ns, :])
                nc.vector.tensor_add(o_sb[:], o_sb[:], bo_bc[:])
                nc.sync.dma_start(out[n0 + ns * P:n0 + (ns + 1) * P, :], o_sb[:])


_NC_CACHE = {}


def _build(has_ba=False):
    if has_ba in _NC_CACHE:
        return _NC_CACHE[has_ba]
    nc = bacc.Bacc("TRN2", target_bir_lowering=False, debug=False,
                   num_devices=NCORES)
    nc.t_audio = nc.dram_tensor("audio", [BL * SEQ, D], F32, kind="ExternalInput")
    nc.t_video = nc.dram_tensor("video", [BL * SEQ, D], F32, kind="ExternalInput")
    nc.t_wa = nc.dram_tensor("w_a", [D, D], F32, kind="ExternalInput")
    nc.t_wv = nc.dram_tensor("w_v", [D, D], F32, kind="ExternalInput")
    nc.t_wo = nc.dram_tensor("w_o", [D, D], F32, kind="ExternalInput")
    nc.t_ba = nc.dram_tensor("b_a", [D], F32, kind="ExternalInput")
    nc.t_bo = nc.dram_tensor("b_o", [D], F32, kind="ExternalInput")
    nc.t_out = nc.dram_tensor("out", [BL * SEQ, D], F32, kind="ExternalOutput")
    with tile.TileContext(nc) as tc:
        with ExitStack() as ctx:
            _body(tc, ctx, has_ba=has_ba)
    nc.compile()
    _NC_CACHE[has_ba] = nc
    return nc


def kernel(audio, video, W_a, b_a, W_v, b_v, W_o, b_o, _trace=False):
    nc = _build(has_ba=bool(np.any(np.asarray(b_a))))
    audio = np.ascontiguousarray(audio, dtype=np.float32)
    video = np.ascontiguousarray(video, dtype=np.float32)
    shared = {
        "w_a": np.ascontiguousarray(W_a, dtype=np.float32),
        "w_v": np.ascontiguousarray(W_v, dtype=np.float32),
        "w_o": np.ascontiguousarray(W_o, dtype=np.float32),
        "b_a": np.ascontiguousarray(b_a, dtype=np.float32),
        "b_o": np.ascontiguousarray(b_o, dtype=np.float32),
    }
    in_maps = []
    for c in range(NCORES):
        sl = slice(c * BL, (c + 1) * BL)
        in_maps.append({
            "audio": audio[sl].reshape(BL * SEQ, D),
            "video": video[sl].reshape(BL * SEQ, D),
            **shared,
        })
    res = run_bass_kernel_spmd(nc, in_maps, core_ids=list(range(NCORES)),
                               trace=_trace)
    out = np.concatenate(
        [res.results[c]["out"].reshape(BL, SEQ, D) for c in range(NCORES)],
        axis=0)
    if _trace:
        kernel.last_exec_time_ns = res.exec_time_ns
        kernel.last_results = res
    return out
